# revision 9
# baseline (speedup 1.0000x reference)
"""Trainium2 kernel for AttentionConstMemory.

Reference computation (B=256, IN=1024, OUT=1024, DEPTH=64, MEM=256):
    query = (inputs @ Wq.T).reshape(B, DEPTH, OUT)          # 34.4 GFLOP
    key   = Wk @ const_mem.reshape(DEPTH, MEM)              # batch-constant
    att   = softmax(einsum('bdo,bdm->bom', query, key) / 8)
    out   = einsum('bom,bdm->bod', att, key)                # (B, OUT, DEPTH)

Sharding: tensor-parallel over OUT across 8 cores (128 columns each).
No collectives needed — each core computes its output slice end to end;
the host concatenates. All-bf16: fp8 variants of the query/logits path
measure 2-4e-2 relative error (softmax weight jitter) — over budget.

Per-core layout (o = this core's 128 output columns, 64 do-tiles of
128 = 2 o-values x 64 depth, processed in pairs tp):
  - Wq host-prepped to (i, o*64+d) as contiguous per-group blocks so
    each weight group is one 2D DMA; query matmul keeps wq stationary
    (128-col LDW, FWL) with xt moving at N=256.
  - logits row-packed: two concurrent K=64 matmuls in row-groups 0/64,
    key2 stationary [64,128], qs moving N=256 -> lps (m, b) per tile.
  - exp: one ACTIVATE per tile (1024 wide), bias -3 (softmax-invariant)
    -> es bf16.
  - einsum-2: es stationary [128,128] (FWL), kt moving: keyT augmented
    with a ones column so the softmax denominator Z drops out of the
    same matmul (Z lands at column 64 of each 128-col PSUM slot).
  - epilogue: reciprocal of Z + one stride-0-broadcast tensor_mul per
    tile straight from PSUM into bf16 og (halves the output DMA).
  - warmup matmuls run on a memset tile (no DMA dependency) so the PE
    HAM clock-gate reaches 2.4GHz during the initial weight-DMA window;
    a dummy exp preloads the ACT spline table at t~0.
"""

import dataclasses

import numpy as np
import ml_dtypes

B = 256
IN_DIM = 1024
OUT_DIM = 1024
DEPTH = 64
MEM = 256
N_CORES = 8
O_PER_CORE = OUT_DIM // N_CORES  # 128
N_TILES = 64                     # query do-tiles per core; each = 2 o values
GROUP_SIZES = [2, 4, 6, 8, 8, 8, 8, 8, 6, 4, 2]
N_WARMUP_MM = 10
BF16 = ml_dtypes.bfloat16
WQ_SCALE = 32.0


def build_nc():
    import concourse.bacc as bacc
    import concourse.mybir as mybir
    from concourse.tile import TileContext

    fp32 = mybir.dt.float32
    bf16 = mybir.dt.bfloat16

    nc = bacc.Bacc(None, target_bir_lowering=False, debug=False)

    xt = nc.declare_dram_parameter("xt", [IN_DIM, B], bf16, isOutput=False)
    # per-group contiguous blocks [p, woff + k*gw + tl*128 + j]
    wq = nc.declare_dram_parameter("wq", [128, 8 * N_TILES * 128], bf16, isOutput=False)
    wkt = nc.declare_dram_parameter("wkt", [DEPTH, DEPTH], fp32, isOutput=False)
    mem = nc.declare_dram_parameter("mem", [DEPTH, MEM], fp32, isOutput=False)
    # out[bc, b128, o_local*64 + d]
    out_d = nc.declare_dram_parameter("out", [2, 128, O_PER_CORE * DEPTH], bf16, isOutput=True)

    Exp = mybir.ActivationFunctionType.Exp

    def bc_ap(ap, offset_delta, dims):
        """Rebuild an AP at offset+delta with explicit [step, count] free dims."""
        return dataclasses.replace(
            ap, offset=ap.offset + offset_delta, ap=[list(ap.ap[0])] + [list(d) for d in dims]
        )

    with TileContext(nc) as tc:
        with (
            tc.tile_pool(name="const", bufs=1) as cpool,
            tc.tile_pool(name="wq", bufs=3) as wpool,
            tc.tile_pool(name="qsb", bufs=3) as qpool_sb,
            tc.tile_pool(name="esb", bufs=3) as epool,
            tc.tile_pool(name="og", bufs=2) as ogpool,
            tc.tile_pool(name="rz", bufs=8) as rzpool,
            tc.tile_pool(name="qps", bufs=2, space="PSUM") as qpool,
            tc.tile_pool(name="lps", bufs=2, space="PSUM") as lpool,
            tc.tile_pool(name="ops", bufs=2, space="PSUM") as opool,
        ):
            # --- PE warmup with zero DMA dependency: memset tile + matmul
            # chain keeps the PE busy so the HAM clock-gate hits 2.4GHz
            # before the first weight group lands. ---
            wrm = cpool.tile([128, 512], bf16)
            nc.vector.memset(wrm[:, :], 0.25)
            warm = qpool.tile([128, 512], fp32, tag="qps")
            for i in range(N_WARMUP_MM):
                nc.tensor.matmul(
                    warm[:, :], wrm[:, 0:128], wrm[:, :],
                    start=(i == 0), stop=(i == N_WARMUP_MM - 1),
                )
            # dummy exp: forces the ACT spline-table load during the DMA window
            bias_t = cpool.tile([128, 1], fp32)
            nc.vector.memset(bias_t[:, :], -3.0)
            edum = cpool.tile([128, 8], bf16)
            nc.scalar.activation(edum[:, :], wrm[:, 0:8], Exp, scale=1.0, bias=bias_t[:, :])

            # --- tiny constants ---
            wkt_sb = cpool.tile([DEPTH, DEPTH], fp32)
            nc.sync.dma_start(out=wkt_sb[:, :], in_=wkt[:, :])
            mem_sb = cpool.tile([DEPTH, MEM], fp32)
            nc.sync.dma_start(out=mem_sb[:, :], in_=mem[:, :])

            xt_sb = cpool.tile([128, 8 * B], bf16)  # [p, k*256+b] = XT[k*128+p, b]
            for k in range(8):
                eng = nc.sync if k % 2 == 0 else nc.scalar
                eng.dma_start(
                    out=xt_sb[:, k * B : (k + 1) * B],
                    in_=xt[k * 128 : (k + 1) * 128, :],
                )

            # --- key2 (128, 256) bf16: key duplicated in both partition halves ---
            # key[d, m] = sum_k Wk[d, k] mem[k, m];  lhsT = wkt (k, d)
            kps = qpool.tile([128, 512], fp32, tag="qps")
            nc.tensor.matmul(kps[0:64, 0:MEM], wkt_sb[:, :], mem_sb[:, :], start=True, stop=True)
            nc.tensor.matmul(kps[64:128, 0:MEM], wkt_sb[:, :], mem_sb[:, :], start=True, stop=True)
            key2 = cpool.tile([128, MEM], bf16)
            nc.vector.tensor_copy(key2[:, :], kps[:, 0:MEM])

            # --- keyT augmented with ones column: kt[mc] (128 m, 65) bf16 ---
            # keyT[m, d] = sum_k mem[k, m] Wk[d, k]; lhsT = mem chunk, rhs = wkt
            ktp = qpool.tile([128, 512], fp32, tag="qps")
            nc.tensor.matmul(ktp[:, 0:DEPTH], mem_sb[:, 0:128], wkt_sb[:, :], start=True, stop=True)
            nc.tensor.matmul(ktp[:, DEPTH : 2 * DEPTH], mem_sb[:, 128:256], wkt_sb[:, :], start=True, stop=True)
            kt = cpool.tile([128, 2 * (DEPTH + 1)], bf16)  # [mc*65 : mc*65+65]
            nc.vector.tensor_copy(kt[:, 0:DEPTH], ktp[:, 0:DEPTH])
            nc.vector.tensor_copy(kt[:, DEPTH + 1 : 2 * DEPTH + 1], ktp[:, DEPTH : 2 * DEPTH])
            nc.vector.memset(kt[:, DEPTH : DEPTH + 1], 1.0)
            nc.vector.memset(kt[:, 2 * DEPTH + 1 : 2 * DEPTH + 2], 1.0)

            t0 = 0
            woff = 0
            for gi, nt in enumerate(GROUP_SIZES):
                gw = nt * 128  # do-columns in this group
                wg = wpool.tile([128, 8 * gw], bf16, tag="wg")  # [p, k*gw + tl*128 + j]
                if gi < 2:  # startup: split across both HWDGE queue families
                    nc.sync.dma_start(out=wg[:, 0 : 4 * gw], in_=wq[:, woff : woff + 4 * gw])
                    nc.scalar.dma_start(
                        out=wg[:, 4 * gw : 8 * gw], in_=wq[:, woff + 4 * gw : woff + 8 * gw]
                    )
                else:
                    nc.sync.dma_start(out=wg[:, :], in_=wq[:, woff : woff + 8 * gw])
                og = ogpool.tile([128, 2 * gw], bf16, tag="og")  # [p, bc*gw + tl*128 + oi*64 + d]
                for tp in range(nt // 2):
                    # ---- query tile pair: psum (128=[o_even d | o_odd d], 512=2x256 b) ----
                    qps = qpool.tile([128, 2 * B], fp32, tag="qps")
                    for half in range(2):
                        tl = tp * 2 + half
                        for k in range(8):
                            nc.tensor.matmul(
                                qps[:, half * B : (half + 1) * B],
                                wg[:, k * gw + tl * 128 : k * gw + tl * 128 + 128],
                                xt_sb[:, k * B : (k + 1) * B],
                                start=(k == 0),
                                stop=(k == 7),
                            )
                    qs = qpool_sb.tile([128, 2 * B], bf16)
                    nc.vector.tensor_copy(qs[:, :], qps[:, :])

                    for half in range(2):
                        tl = tp * 2 + half
                        qoff = half * B
                        # ---- logits: lps[m, (oi*2+mc)*256 + b]; two K=64 matmuls
                        # run concurrently in row-groups 0/64 ----
                        lps = lpool.tile([128, 4 * B], fp32, tag="lps")
                        for mc in range(2):
                            for oi in range(2):
                                pb = 64 * oi
                                nc.tensor.matmul(
                                    lps[:, (oi * 2 + mc) * B : (oi * 2 + mc + 1) * B],
                                    key2[pb : pb + 64, mc * 128 : (mc + 1) * 128],
                                    qs[pb : pb + 64, qoff : qoff + B],
                                    start=True,
                                    stop=True,
                                )
                        # ---- exp (one ACT op per tile) ----
                        es = epool.tile([128, 4 * B], bf16, tag="es")
                        nc.scalar.activation(
                            es[:, :], lps[:, :], Exp,
                            scale=float(DEPTH**-0.5 / WQ_SCALE), bias=bias_t[:, :],
                        )

                        # ---- einsum-2 + Z: ops[b, j*128 : j*128+65], j = oi*2+bcc ----
                        ops = opool.tile([128, 512], fp32, tag="ops")
                        for oi in range(2):
                            for bcc in range(2):
                                j = oi * 2 + bcc
                                for mc in range(2):
                                    nc.tensor.matmul(
                                        ops[:, j * 128 : j * 128 + 65],
                                        es[:, oi * 2 * B + mc * B + bcc * 128 : oi * 2 * B + mc * B + bcc * 128 + 128],
                                        kt[:, mc * 65 : mc * 65 + 65],
                                        start=(mc == 0),
                                        stop=(mc == 1),
                                    )
                        # ---- softmax divide: rz = 1/Z, then one broadcast mul ----
                        rz = rzpool.tile([128, 4], fp32, tag="rz")
                        nc.vector.reciprocal(rz[:, :], ops[:, 64 : 512 : 128])
                        in0 = bc_ap(ops[:, :], 0, [[2 * 128, 2], [128, 2], [1, DEPTH]])
                        in1 = bc_ap(rz[:, :], 0, [[2, 2], [1, 2], [0, DEPTH]])
                        outp = bc_ap(og[:, :], tl * 128, [[DEPTH, 2], [gw, 2], [1, DEPTH]])
                        nc.vector.tensor_mul(outp, in0, in1)
                    if gi >= len(GROUP_SIZES) - 2:
                        # tail groups: flush per tile-pair so the final DMA is tiny
                        for bcc in range(2):
                            nc.sync.dma_start(
                                out=out_d[bcc, :, (t0 + tp * 2) * 128 : (t0 + tp * 2) * 128 + 256],
                                in_=og[:, bcc * gw + tp * 256 : bcc * gw + tp * 256 + 256],
                            )
                if gi < len(GROUP_SIZES) - 2:
                    for bcc in range(2):
                        nc.sync.dma_start(
                            out=out_d[bcc, :, t0 * 128 : t0 * 128 + gw],
                            in_=og[:, bcc * gw : (bcc + 1) * gw],
                        )
                t0 += nt
                woff += 8 * gw
    nc.finalize()
    return nc


def prep_in_maps(inputs, const_mem, Wq, Wk):
    xt = np.ascontiguousarray(np.asarray(inputs).T).astype(BF16)
    wkt = np.ascontiguousarray(Wk.T).astype(np.float32)
    mem = np.ascontiguousarray(const_mem.reshape(DEPTH, MEM)).astype(np.float32)
    # (d, o, i) -> (i, o*64+d) per core; x32 (fp32-exact) so the exp scale is 1/256
    wqt = (Wq.reshape(DEPTH, OUT_DIM, IN_DIM) * WQ_SCALE).transpose(2, 1, 0)
    in_maps = []
    for c in range(N_CORES):
        wq_c = np.ascontiguousarray(
            wqt[:, c * O_PER_CORE : (c + 1) * O_PER_CORE, :]
        ).reshape(IN_DIM, N_TILES * 128)
        # per-group contiguous blocks: [p, woff + k*gw + tl*128 + j]
        blocks = []
        t0 = 0
        for nt in GROUP_SIZES:
            gw = nt * 128
            blk = wq_c[:, t0 * 128 : t0 * 128 + gw].reshape(8, 128, gw)  # [k, p, col]
            blocks.append(blk.transpose(1, 0, 2).reshape(128, 8 * gw))
            t0 += nt
        in_maps.append({
            "xt": xt,
            "wq": np.ascontiguousarray(np.concatenate(blocks, axis=1)).astype(BF16),
            "wkt": wkt,
            "mem": mem,
        })
    return in_maps


def gather_output(results):
    out = np.empty((B, OUT_DIM, DEPTH), dtype=np.float32)
    for c in range(N_CORES):
        oc = results[c]["out"]  # (2, 128, 8192) bf16
        out[:, c * O_PER_CORE : (c + 1) * O_PER_CORE, :] = oc.reshape(
            B, O_PER_CORE, DEPTH
        ).astype(np.float32)
    return out


def kernel(inputs, const_mem, Wq, Wk):
    from concourse.bass_utils import run_bass_kernel_spmd

    nc = build_nc()
    in_maps = prep_in_maps(
        np.asarray(inputs), np.asarray(const_mem), np.asarray(Wq), np.asarray(Wk)
    )
    res = run_bass_kernel_spmd(nc, in_maps, core_ids=list(range(N_CORES)))
    return gather_output(res.results)


# revision 12
# speedup vs baseline: 1.0441x; 1.0441x over previous
"""Trainium2 kernel for AttentionConstMemory.

Reference computation (B=256, IN=1024, OUT=1024, DEPTH=64, MEM=256):
    query = (inputs @ Wq.T).reshape(B, DEPTH, OUT)          # 34.4 GFLOP
    key   = Wk @ const_mem.reshape(DEPTH, MEM)              # batch-constant
    att   = softmax(einsum('bdo,bdm->bom', query, key) / 8)
    out   = einsum('bom,bdm->bod', att, key)                # (B, OUT, DEPTH)

Sharding: tensor-parallel over OUT across 8 cores (128 columns each).
No collectives needed — each core computes its output slice end to end;
the host concatenates. All-bf16: fp8 variants of the query/logits path
measure 2-4e-2 relative error (softmax weight jitter) — over budget.

Per-core layout (o = this core's 128 output columns, 64 do-tiles of
128 = 2 o-values x 64 depth, processed in pairs tp):
  - Wq host-prepped to (i, o*64+d) as contiguous per-group blocks so
    each weight group is one 2D DMA; query matmul keeps wq stationary
    (128-col LDW, FWL) with xt moving at N=256.
  - logits row-packed: two concurrent K=64 matmuls in row-groups 0/64,
    key2 stationary [64,128], qs moving N=256 -> lps (m, b) per tile.
  - exp: one ACTIVATE per tile (1024 wide), bias -3 (softmax-invariant)
    -> es bf16.
  - einsum-2: es stationary [128,128] (FWL), kt moving: keyT augmented
    with a ones column so the softmax denominator Z drops out of the
    same matmul (Z lands at column 64 of each 128-col PSUM slot).
  - epilogue: reciprocal of Z + one stride-0-broadcast tensor_mul per
    tile straight from PSUM into bf16 og (halves the output DMA).
  - warmup matmuls run on a memset tile (no DMA dependency) so the PE
    HAM clock-gate reaches 2.4GHz during the initial weight-DMA window;
    a dummy exp preloads the ACT spline table at t~0.
"""

import dataclasses

import numpy as np
import ml_dtypes

B = 256
IN_DIM = 1024
OUT_DIM = 1024
DEPTH = 64
MEM = 256
N_CORES = 8
O_PER_CORE = OUT_DIM // N_CORES  # 128
N_TILES = 64                     # query do-tiles per core; each = 2 o values
GROUP_SIZES = [2, 6, 8, 8, 8, 8, 8, 8, 6, 2]
N_WARMUP_MM = 22
BF16 = ml_dtypes.bfloat16
WQ_SCALE = 32.0


def build_nc():
    import concourse.bacc as bacc
    import concourse.mybir as mybir
    from concourse.tile import TileContext

    fp32 = mybir.dt.float32
    bf16 = mybir.dt.bfloat16

    nc = bacc.Bacc(None, target_bir_lowering=False, debug=False)

    xt = nc.declare_dram_parameter("xt", [IN_DIM, B], bf16, isOutput=False)
    # per-group contiguous blocks [p, woff + k*gw + tl*128 + j]
    wq = nc.declare_dram_parameter("wq", [128, 8 * N_TILES * 128], bf16, isOutput=False)
    wkt = nc.declare_dram_parameter("wkt", [DEPTH, DEPTH], fp32, isOutput=False)
    mem = nc.declare_dram_parameter("mem", [DEPTH, MEM], fp32, isOutput=False)
    # out[bc, b128, o_local*64 + d]
    out_d = nc.declare_dram_parameter("out", [2, 128, O_PER_CORE * DEPTH], bf16, isOutput=True)

    Exp = mybir.ActivationFunctionType.Exp

    def bc_ap(ap, offset_delta, dims):
        """Rebuild an AP at offset+delta with explicit [step, count] free dims."""
        return dataclasses.replace(
            ap, offset=ap.offset + offset_delta, ap=[list(ap.ap[0])] + [list(d) for d in dims]
        )

    with TileContext(nc) as tc:
        with (
            tc.tile_pool(name="const", bufs=1) as cpool,
            tc.tile_pool(name="wq", bufs=3) as wpool,
            tc.tile_pool(name="qsb", bufs=3) as qpool_sb,
            tc.tile_pool(name="esb", bufs=3) as epool,
            tc.tile_pool(name="og", bufs=2) as ogpool,
            tc.tile_pool(name="rz", bufs=8) as rzpool,
            tc.tile_pool(name="qps", bufs=2, space="PSUM") as qpool,
            tc.tile_pool(name="lps", bufs=2, space="PSUM") as lpool,
            tc.tile_pool(name="ops", bufs=2, space="PSUM") as opool,
        ):
            # --- PE warmup with zero DMA dependency: memset tile + matmul
            # chain keeps the PE busy so the HAM clock-gate hits 2.4GHz
            # before the first weight group lands. ---
            wrm = cpool.tile([128, 512], bf16)
            nc.vector.memset(wrm[:, :], 0.25)
            warm = qpool.tile([128, 512], fp32, tag="qps")
            for i in range(N_WARMUP_MM):
                nc.tensor.matmul(
                    warm[:, :], wrm[:, 0:128], wrm[:, :],
                    start=(i == 0), stop=(i == N_WARMUP_MM - 1),
                )
            # dummy exp: forces the ACT spline-table load during the DMA window
            bias_t = cpool.tile([128, 1], fp32)
            nc.vector.memset(bias_t[:, :], -3.0)
            edum = cpool.tile([128, 8], bf16)
            nc.scalar.activation(edum[:, :], wrm[:, 0:8], Exp, scale=1.0, bias=bias_t[:, :])

            # --- tiny constants ---
            wkt_sb = cpool.tile([DEPTH, DEPTH], fp32)
            nc.sync.dma_start(out=wkt_sb[:, :], in_=wkt[:, :])
            mem_sb = cpool.tile([DEPTH, MEM], fp32)
            nc.sync.dma_start(out=mem_sb[:, :], in_=mem[:, :])

            xt_sb = cpool.tile([128, 8 * B], bf16)  # [p, k*256+b] = XT[k*128+p, b]
            for k in range(8):
                eng = nc.sync if k % 2 == 0 else nc.scalar
                eng.dma_start(
                    out=xt_sb[:, k * B : (k + 1) * B],
                    in_=xt[k * 128 : (k + 1) * 128, :],
                )

            # --- key2 (128, 256) bf16: key duplicated in both partition halves ---
            # key[d, m] = sum_k Wk[d, k] mem[k, m];  lhsT = wkt (k, d)
            kps = qpool.tile([128, 512], fp32, tag="qps")
            nc.tensor.matmul(kps[0:64, 0:MEM], wkt_sb[:, :], mem_sb[:, :], start=True, stop=True)
            nc.tensor.matmul(kps[64:128, 0:MEM], wkt_sb[:, :], mem_sb[:, :], start=True, stop=True)
            key2 = cpool.tile([128, MEM], bf16)
            nc.vector.tensor_copy(key2[:, :], kps[:, 0:MEM])

            # --- keyT augmented with ones column: kt[mc] (128 m, 65) bf16 ---
            # keyT[m, d] = sum_k mem[k, m] Wk[d, k]; lhsT = mem chunk, rhs = wkt
            ktp = qpool.tile([128, 512], fp32, tag="qps")
            nc.tensor.matmul(ktp[:, 0:DEPTH], mem_sb[:, 0:128], wkt_sb[:, :], start=True, stop=True)
            nc.tensor.matmul(ktp[:, DEPTH : 2 * DEPTH], mem_sb[:, 128:256], wkt_sb[:, :], start=True, stop=True)
            kt = cpool.tile([128, 2 * (DEPTH + 1)], bf16)  # [mc*65 : mc*65+65]
            nc.vector.tensor_copy(kt[:, 0:DEPTH], ktp[:, 0:DEPTH])
            nc.vector.tensor_copy(kt[:, DEPTH + 1 : 2 * DEPTH + 1], ktp[:, DEPTH : 2 * DEPTH])
            nc.vector.memset(kt[:, DEPTH : DEPTH + 1], 1.0)
            nc.vector.memset(kt[:, 2 * DEPTH + 1 : 2 * DEPTH + 2], 1.0)

            t0 = 0
            woff = 0
            for gi, nt in enumerate(GROUP_SIZES):
                gw = nt * 128  # do-columns in this group
                wg = wpool.tile([128, 8 * gw], bf16, tag="wg")  # [p, k*gw + tl*128 + j]
                # split each group across both HWDGE queue families
                nc.sync.dma_start(out=wg[:, 0 : 4 * gw], in_=wq[:, woff : woff + 4 * gw])
                nc.scalar.dma_start(
                    out=wg[:, 4 * gw : 8 * gw], in_=wq[:, woff + 4 * gw : woff + 8 * gw]
                )
                og = ogpool.tile([128, 2 * gw], bf16, tag="og")  # [p, bc*gw + tl*128 + oi*64 + d]
                for tp in range(nt // 2):
                    # ---- query tile pair: psum (128=[o_even d | o_odd d], 512=2x256 b) ----
                    qps = qpool.tile([128, 2 * B], fp32, tag="qps")
                    for half in range(2):
                        tl = tp * 2 + half
                        for k in range(8):
                            nc.tensor.matmul(
                                qps[:, half * B : (half + 1) * B],
                                wg[:, k * gw + tl * 128 : k * gw + tl * 128 + 128],
                                xt_sb[:, k * B : (k + 1) * B],
                                start=(k == 0),
                                stop=(k == 7),
                            )
                    qs = qpool_sb.tile([128, 2 * B], bf16)
                    nc.vector.tensor_copy(qs[:, :], qps[:, :])

                    for half in range(2):
                        tl = tp * 2 + half
                        qoff = half * B
                        # ---- logits: lps[m, (oi*2+mc)*256 + b]; two K=64 matmuls
                        # run concurrently in row-groups 0/64 ----
                        lps = lpool.tile([128, 4 * B], fp32, tag="lps")
                        for mc in range(2):
                            for oi in range(2):
                                pb = 64 * oi
                                nc.tensor.matmul(
                                    lps[:, (oi * 2 + mc) * B : (oi * 2 + mc + 1) * B],
                                    key2[pb : pb + 64, mc * 128 : (mc + 1) * 128],
                                    qs[pb : pb + 64, qoff : qoff + B],
                                    start=True,
                                    stop=True,
                                )
                        # ---- exp (one ACT op per tile) ----
                        es = epool.tile([128, 4 * B], bf16, tag="es")
                        nc.scalar.activation(
                            es[:, :], lps[:, :], Exp,
                            scale=float(DEPTH**-0.5 / WQ_SCALE), bias=bias_t[:, :],
                        )

                        # ---- einsum-2 + Z: ops[b, j*128 : j*128+65], j = oi*2+bcc ----
                        ops = opool.tile([128, 512], fp32, tag="ops")
                        for oi in range(2):
                            for bcc in range(2):
                                j = oi * 2 + bcc
                                for mc in range(2):
                                    nc.tensor.matmul(
                                        ops[:, j * 128 : j * 128 + 65],
                                        es[:, oi * 2 * B + mc * B + bcc * 128 : oi * 2 * B + mc * B + bcc * 128 + 128],
                                        kt[:, mc * 65 : mc * 65 + 65],
                                        start=(mc == 0),
                                        stop=(mc == 1),
                                    )
                        # ---- softmax divide: rz = 1/Z, then one broadcast mul ----
                        rz = rzpool.tile([128, 4], fp32, tag="rz")
                        nc.vector.reciprocal(rz[:, :], ops[:, 64 : 512 : 128])
                        in0 = bc_ap(ops[:, :], 0, [[2 * 128, 2], [128, 2], [1, DEPTH]])
                        in1 = bc_ap(rz[:, :], 0, [[2, 2], [1, 2], [0, DEPTH]])
                        outp = bc_ap(og[:, :], tl * 128, [[DEPTH, 2], [gw, 2], [1, DEPTH]])
                        nc.vector.tensor_mul(outp, in0, in1)
                    if gi >= len(GROUP_SIZES) - 2:
                        # tail groups: flush per tile-pair so the final DMA is tiny
                        for bcc in range(2):
                            nc.sync.dma_start(
                                out=out_d[bcc, :, (t0 + tp * 2) * 128 : (t0 + tp * 2) * 128 + 256],
                                in_=og[:, bcc * gw + tp * 256 : bcc * gw + tp * 256 + 256],
                            )
                if gi < len(GROUP_SIZES) - 2:
                    for bcc in range(2):
                        nc.sync.dma_start(
                            out=out_d[bcc, :, t0 * 128 : t0 * 128 + gw],
                            in_=og[:, bcc * gw : (bcc + 1) * gw],
                        )
                t0 += nt
                woff += 8 * gw
    nc.finalize()
    return nc


def prep_in_maps(inputs, const_mem, Wq, Wk):
    xt = np.ascontiguousarray(np.asarray(inputs).T).astype(BF16)
    wkt = np.ascontiguousarray(Wk.T).astype(np.float32)
    mem = np.ascontiguousarray(const_mem.reshape(DEPTH, MEM)).astype(np.float32)
    # (d, o, i) -> (i, o*64+d) per core; x32 (fp32-exact) so the exp scale is 1/256
    wqt = (Wq.reshape(DEPTH, OUT_DIM, IN_DIM) * WQ_SCALE).transpose(2, 1, 0)
    in_maps = []
    for c in range(N_CORES):
        wq_c = np.ascontiguousarray(
            wqt[:, c * O_PER_CORE : (c + 1) * O_PER_CORE, :]
        ).reshape(IN_DIM, N_TILES * 128)
        # per-group contiguous blocks: [p, woff + k*gw + tl*128 + j]
        blocks = []
        t0 = 0
        for nt in GROUP_SIZES:
            gw = nt * 128
            blk = wq_c[:, t0 * 128 : t0 * 128 + gw].reshape(8, 128, gw)  # [k, p, col]
            blocks.append(blk.transpose(1, 0, 2).reshape(128, 8 * gw))
            t0 += nt
        in_maps.append({
            "xt": xt,
            "wq": np.ascontiguousarray(np.concatenate(blocks, axis=1)).astype(BF16),
            "wkt": wkt,
            "mem": mem,
        })
    return in_maps


def gather_output(results):
    out = np.empty((B, OUT_DIM, DEPTH), dtype=np.float32)
    for c in range(N_CORES):
        oc = results[c]["out"]  # (2, 128, 8192) bf16
        out[:, c * O_PER_CORE : (c + 1) * O_PER_CORE, :] = oc.reshape(
            B, O_PER_CORE, DEPTH
        ).astype(np.float32)
    return out


def kernel(inputs, const_mem, Wq, Wk):
    from concourse.bass_utils import run_bass_kernel_spmd

    nc = build_nc()
    in_maps = prep_in_maps(
        np.asarray(inputs), np.asarray(const_mem), np.asarray(Wq), np.asarray(Wk)
    )
    res = run_bass_kernel_spmd(nc, in_maps, core_ids=list(range(N_CORES)))
    return gather_output(res.results)
